# revision 26
# baseline (speedup 1.0000x reference)
"""DiverseExpertLoss on 8 Trainium2 NeuronCores (pure data-parallel over batch).

Device computes, per batch row b (natural layout, rows on partitions):
  S1 = sum_c exp(e1[b,c])             S2 = sum_c exp(e2[b,c] + c2[c])
  S3 = sum_c exp(e3[b,c] + c3[c])     Zt = sum_c exp(o[b,c]/2)
  Zs = sum_c exp(y[b,c]/2)            td = sum_c exp(o[b,c]/2) * (o[b,c]-y[b,c])
  m_o = max_c o[b,c]
  y[b,:] = output[b,:] / counts[t_b]        (the expert_sum_logits output)
plus the class-aggregated matrices A_k = one_hot(target)^T @ e_k in PSUM
(bf16 inputs, fp32 accumulate).  The host does the O(B)/O(C^2) assembly
(logs, means, the argmax==target mask via a gather + compare against the
device row-max, Frobenius products with the label table) in float64.
"""

import threading

import numpy as np

B = 65536
C = 100
NCORES = 8
BS = B // NCORES          # rows per core = 8192
NST = 4                   # supertiles per core
G = 16                    # 128-row blocks per supertile
ROWS_ST = 128 * G         # 2048 rows per supertile
ALPHA = 0.01
TAU = 2.0
TEMP = 2.0
EPS = 1e-9

_lock = threading.Lock()
_cache: dict = {}


def _split_waits(nc, maxw=1):
    """walrus codegen allows only a small number of sync-wait commands per
    instruction; offload extras onto preceding same-engine NoOps."""
    import concourse.mybir as mybir
    nid = 0
    for fn in nc.m.functions:
        for blk in fn.blocks:
            out = []
            changed = False
            for inst in blk.instructions:
                si = inst.sync_info
                if si is not None and si.on_wait is not None \
                        and len(si.on_wait) > maxw:
                    waits = list(si.on_wait)
                    for w in waits[maxw:]:
                        nid += 1
                        nop = mybir.InstNoOp(name=f"WSPLIT-{nid}")
                        nop.engine = inst.engine
                        nop.sync_info = mybir.SyncInfo(on_wait=[w], on_update=[])
                        out.append(nop)
                    inst.sync_info = mybir.SyncInfo(
                        on_wait=waits[:maxw],
                        on_update=list(si.on_update or []))
                    changed = True
                out.append(inst)
            if changed:
                blk.instructions = out


def _build_nc():
    import concourse.bass as bass
    import concourse.tile as tile
    import concourse.mybir as mybir
    from contextlib import ExitStack

    Op = mybir.AluOpType
    Act = mybir.ActivationFunctionType
    Ax = mybir.AxisListType
    f32 = mybir.dt.float32
    bf16 = mybir.dt.bfloat16

    nc = bass.Bass("TRN2", target_bir_lowering=False, debug=False)

    fp8 = mybir.dt.float8e4
    # packed fp8: [st, p, {e1,e2,e3,onehot}, g, c]  (feeds A_k matmuls only)
    ep = nc.dram_tensor("ep", [NST, 128, 4, G, C], fp8, kind="ExternalInput").ap()
    op_ = nc.dram_tensor("op", [NST, 128, G, C], f32, kind="ExternalInput").ap()
    u = nc.dram_tensor("u", [NST, 128, G, C], f32, kind="ExternalInput").ap()
    eT1 = nc.dram_tensor("eT1", [C, BS], bf16, kind="ExternalInput").ap()
    eT2 = nc.dram_tensor("eT2", [C, BS], bf16, kind="ExternalInput").ap()
    eT3 = nc.dram_tensor("eT3", [C, BS], bf16, kind="ExternalInput").ap()
    iv = nc.dram_tensor("iv", [NST, 128, G], f32, kind="ExternalInput").ap()
    cb = nc.dram_tensor("cb", [C, 3], f32, kind="ExternalInput").ap()

    y = nc.dram_tensor("y", [NST, 128, G, C], f32, kind="ExternalOutput").ap()
    st_out = nc.dram_tensor("stats", [128, 8, NST * G], f32, kind="ExternalOutput").ap()
    a_out = nc.dram_tensor("aout", [3, C, C], f32, kind="ExternalOutput").ap()
    lse_out = nc.dram_tensor("lse", [3, 128, NST * G], f32, kind="ExternalOutput").ap()

    with ExitStack() as ctx:
        tc = ctx.enter_context(tile.TileContext(nc))
        ins = ctx.enter_context(tc.tile_pool(name="ins", bufs=4))
        sc = ctx.enter_context(tc.tile_pool(name="sc", bufs=3))
        sm = ctx.enter_context(tc.tile_pool(name="sm", bufs=2))
        one = ctx.enter_context(tc.tile_pool(name="one", bufs=1))
        scr = ctx.enter_context(tc.tile_pool(name="scr", bufs=4))
        ps = ctx.enter_context(tc.tile_pool(name="ps", bufs=1, space="PSUM"))

        cbt = one.tile([C, 3], f32)
        nc.sync.dma_start(out=cbt[:], in_=cb)
        ones_bf = one.tile([C, 1], bf16)
        nc.vector.memset(ones_bf[:], 1.0)

        stats = one.tile([128, 8, NST * G], f32)
        a_ps = [ps.tile([C, C], f32, tag=f"a{k}", name=f"a{k}") for k in range(3)]
        lse_ps = [ps.tile([128, NST * G], f32, tag=f"l{k}", name=f"l{k}")
                  for k in range(3)]
        eT_all = (eT1, eT2, eT3)
        eT_cur = [None, None, None]

        import concourse.bass as bass_mod

        for st in range(NST):
            otl = ins.tile([128, G, C], f32, tag="otl")
            nc.sync.dma_start(out=otl[:], in_=op_[st])
            utl = ins.tile([128, G, C], f32, tag="utl")
            nc.scalar.dma_start(out=utl[:], in_=u[st])
            ot, ut = otl[:], utl[:]
            ivt = sm.tile([128, G], f32, tag="iv")
            nc.sync.dma_start(out=ivt[:], in_=iv[st])
            ept = ins.tile([128, 4, G, C], fp8, tag="ep")
            e1t, e2t, e3t, oht = (ept[:, 0], ept[:, 1], ept[:, 2], ept[:, 3])
            nc.sync.dma_start(out=ept[:], in_=ep[st])

            sl = slice(st * G, (st + 1) * G)

            tt = sc.tile([128, G, C], f32, tag="tt")
            nc.scalar.activation(tt[:], ot, Act.Exp, scale=0.5)

            # transposed experts, quarter-shard per supertile (loads
            # prefetched one supertile ahead): lse via ACT bias-exp + PE
            # per-128-row column-sum matmuls (classes on partitions)
            QB = BS // NST
            if st == 0:
                for k in range(3):
                    t0_ = ins.tile([C, QB], bf16, tag=f"eT{k}",
                                   name=f"eTt{k}_p", bufs=2)
                    nc.sync.dma_start(out=t0_[:], in_=eT_all[k][:, 0:QB])
                    eT_cur[k] = t0_
            for k in range(3):
                eTt = eT_cur[k]
                if st + 1 < NST:
                    nxt = ins.tile([C, QB], bf16, tag=f"eT{k}",
                                   name=f"eTt{k}", bufs=2)
                    nc.sync.dma_start(
                        out=nxt[:],
                        in_=eT_all[k][:, (st + 1) * QB:(st + 2) * QB])
                    eT_cur[k] = nxt
                xT = sc.tile([C, QB], bf16, tag=f"xT{k}", name=f"xT{k}",
                             bufs=2)
                if k == 0:
                    nc.scalar.activation(xT[:], eTt[:], Act.Exp)
                else:
                    nc.scalar.activation(xT[:], eTt[:], Act.Exp,
                                         bias=cbt[:, k:k + 1])
                for g2 in range(QB // 128):
                    j = st * (QB // 128) + g2
                    nc.tensor.matmul(
                        lse_ps[k][:, j:j + 1],
                        lhsT=xT[:, g2 * 128:(g2 + 1) * 128], rhs=ones_bf[:],
                        start=True, stop=True)

            # y = u * inv_num  (inv_num broadcast along the class dim) - GpSimd
            iva = ivt[:]
            iv_b = bass_mod.AP(tensor=iva.tensor, offset=iva.offset,
                               ap=[*iva.ap, [0, C]])
            yt = sc.tile([128, G, C], f32, tag="yt")
            nc.gpsimd.tensor_tensor(out=yt[:], in0=ut, in1=iv_b, op=Op.mult)
            nc.sync.dma_start(out=y[st], in_=yt[:])

            ey = sc.tile([128, G, C], f32, tag="ey")
            nc.scalar.activation(ey[:], yt[:], Act.Exp, scale=0.5)
            dtl = sc.tile([128, G, C], f32, tag="dtl", bufs=1)
            nc.vector.tensor_tensor(out=dtl[:], in0=ot, in1=yt[:],
                                    op=Op.subtract)

            # plain per-row reductions (3D AP: reduce innermost C)
            nc.vector.tensor_reduce(out=stats[:, 3, sl], in_=tt[:], axis=Ax.X, op=Op.add)
            nc.vector.tensor_reduce(out=stats[:, 4, sl], in_=ey[:], axis=Ax.X, op=Op.add)
            nc.vector.tensor_reduce(out=stats[:, 7, sl], in_=ot, axis=Ax.X, op=Op.max)

            for g in range(G):
                col = slice(st * G + g, st * G + g + 1)
                scrap = scr.tile([128, C], f32, tag="scrap")
                nc.vector.scalar_tensor_tensor(
                    out=scrap[:], in0=tt[:, g, :], scalar=1.0, in1=dtl[:, g, :],
                    op0=Op.mult, op1=Op.mult, accum_out=stats[:, 5, col])

                first = (st == 0 and g == 0)
                last = (st == NST - 1 and g == G - 1)
                nc.tensor.matmul(a_ps[0][:], lhsT=oht[:, g, :], rhs=e1t[:, g, :],
                                 start=first, stop=last)
                nc.tensor.matmul(a_ps[1][:], lhsT=oht[:, g, :], rhs=e2t[:, g, :],
                                 start=first, stop=last)
                nc.tensor.matmul(a_ps[2][:], lhsT=oht[:, g, :], rhs=e3t[:, g, :],
                                 start=first, stop=last)

        for k in range(3):
            a_sb = sm.tile([C, C], f32, tag="asb")
            nc.scalar.copy(out=a_sb[:], in_=a_ps[k][:])
            nc.sync.dma_start(out=a_out[k], in_=a_sb[:])
            l_sb = sm.tile([128, NST * G], f32, tag="lsb", name=f"lsb{k}")
            nc.scalar.copy(out=l_sb[:], in_=lse_ps[k][:])
            nc.sync.dma_start(out=lse_out[k], in_=l_sb[:])
        nc.sync.dma_start(out=st_out, in_=stats[:])

    return nc


def _get_nc():
    # hardware path: wait-split applied (walrus codegen requirement);
    # CoreSim users should call _build_nc() directly.
    with _lock:
        if "nc" not in _cache:
            nc = _build_nc()
            _split_waits(nc)
            _cache["nc"] = nc
        return _cache["nc"]


def _make_in_maps(expert1_logits, expert2_logits, expert3_logits, old_pred,
                  output, target, prior):
    import ml_dtypes
    f32 = np.float32
    bf = ml_dtypes.bfloat16
    f8 = ml_dtypes.float8_e4m3
    e1 = np.asarray(expert1_logits, dtype=f32).astype(bf)
    e2 = np.asarray(expert2_logits, dtype=f32).astype(bf)
    e3 = np.asarray(expert3_logits, dtype=f32).astype(bf)
    op_ = np.ascontiguousarray(old_pred, dtype=f32)
    u = np.ascontiguousarray(output, dtype=f32)
    tgt = np.asarray(target).astype(np.int64)
    prior = np.asarray(prior, dtype=f32)

    counts = np.bincount(tgt, minlength=C).astype(f32)
    inv_num = (1.0 / counts.astype(np.float64)).astype(f32)[tgt]  # [B]
    onehot = np.zeros((B, C), dtype=f8)
    onehot[np.arange(B), tgt] = 1

    # per-class lse offsets: c1 = 0, c2 = log(prior+EPS),
    # c3 = c2 - TAU*log(inv_prior+EPS), inv_prior = mirrored-rank prior
    p64 = prior.astype(np.float64)
    idx0 = np.argsort(p64, kind="stable")
    value = p64[idx0]
    idx1 = np.argsort(idx0, kind="stable")
    inv_prior = value[C - 1 - idx1]
    c2 = np.log(p64 + EPS)
    c3 = c2 - TAU * np.log(inv_prior + EPS)
    cb = np.stack([np.zeros(C), c2, c3], axis=1).astype(f32)  # [C, 3]

    # host side of the KL mask: o[b, target_b] (device supplies the row max)
    g_o = np.ascontiguousarray(op_[np.arange(B), tgt])

    ep_full = np.stack(
        [e1.astype(f8).reshape(NCORES, NST, 128, G, C),
         e2.astype(f8).reshape(NCORES, NST, 128, G, C),
         e3.astype(f8).reshape(NCORES, NST, 128, G, C),
         onehot.reshape(NCORES, NST, 128, G, C)],
        axis=3)  # [cores, NST, 128, 4, G, C]
    in_maps = []
    for m in range(NCORES):
        s = slice(m * BS, (m + 1) * BS)
        in_maps.append({
            "ep": ep_full[m],
            "op": op_[s].reshape(NST, 128, G, C),
            "u": u[s].reshape(NST, 128, G, C),
            "iv": inv_num[s].reshape(NST, 128, G),
            "eT1": np.ascontiguousarray(e1[s].T),
            "eT2": np.ascontiguousarray(e2[s].T),
            "eT3": np.ascontiguousarray(e3[s].T),
            "cb": cb,
        })
    aux = {"counts": counts, "target": tgt, "prior": prior,
           "inv_prior": inv_prior, "p64": p64, "g_o": g_o}
    return in_maps, aux


def _run_device(in_maps, trace=False):
    from concourse.bass_utils import run_bass_kernel_spmd
    nc = _get_nc()
    return run_bass_kernel_spmd(nc, in_maps, list(range(NCORES)), trace=trace)


def _stats_rows(stats_core):
    # stats_core: [128, 8, NST*G] -> [8, BS] with row = st*2048 + p*16 + g
    a = stats_core.reshape(128, 8, NST, G)
    return np.transpose(a, (1, 2, 0, 3)).reshape(8, BS)


def _assemble(results, aux, cos_feature):
    f64 = np.float64
    counts = aux["counts"].astype(f64)
    p64 = aux["p64"]

    # per-row stats across all cores
    allst = np.concatenate([_stats_rows(r["stats"]) for r in results], axis=1)
    _, _, _, Zt, Zs, td, _unused, m_o = [allst[i].astype(f64) for i in range(8)]
    # lse sums: [3, 128, 64], entry (m, j) is shard row j*128+m
    Sall = np.concatenate(
        [r["lse"].transpose(0, 2, 1).reshape(3, BS).astype(f64)
         for r in results], axis=1)
    S1, S2, S3 = Sall[0], Sall[1], Sall[2]

    A = np.zeros((3, C, C), dtype=f64)
    for r in results:
        A += r["aout"].astype(f64)

    # label table and per-class constants (float64)
    cosf = np.asarray(cos_feature, dtype=np.float32).astype(f64)
    sim = np.exp(cosf - cosf.max(axis=1, keepdims=True))
    sim /= sim.sum(axis=1, keepdims=True)
    L = (1.0 - ALPHA) * np.eye(C) + ALPHA * sim
    c1 = np.zeros(C)
    c2 = np.log(p64 + EPS)
    c3 = c2 - TAU * np.log(aux["inv_prior"] + EPS)

    loss = 0.0
    for k, ck in enumerate((c1, c2, c3)):
        Sk = (S1, S2, S3)[k]
        lse_mean = np.log(Sk).mean()
        dotsum = (L * A[k]).sum() + (counts * (L @ ck)).sum()
        loss += lse_mean - dotsum / B

    mask = (aux["g_o"].astype(f64) == m_o)
    n_sel = mask.sum()
    kl_per = td / (2.0 * Zt) - np.log(Zt) + np.log(Zs)
    if n_sel > 0:
        kl = (kl_per * mask).sum() / max(n_sel, 1.0) * (TEMP ** 2) * 3.0
    else:
        kl = 0.0

    y_full = np.concatenate(
        [r["y"].reshape(BS, C) for r in results], axis=0)
    return (np.float32(loss), np.float32(kl), np.ascontiguousarray(y_full))


def kernel(output_logits=None, target=None, cos_feature=None, old_pred=None,
           expert1_logits=None, expert2_logits=None, expert3_logits=None,
           output=None, prior=None, epoch=None, **_ignored):
    in_maps, aux = _make_in_maps(expert1_logits, expert2_logits,
                                 expert3_logits, old_pred, output, target,
                                 prior)
    res = _run_device(in_maps)
    return _assemble(res.results, aux, cos_feature)


# revision 27
# speedup vs baseline: 1.0302x; 1.0302x over previous
"""DiverseExpertLoss on 8 Trainium2 NeuronCores (pure data-parallel over batch).

Device computes, per batch row b (natural layout, rows on partitions):
  S1 = sum_c exp(e1[b,c])             S2 = sum_c exp(e2[b,c] + c2[c])
  S3 = sum_c exp(e3[b,c] + c3[c])     Zt = sum_c exp(o[b,c]/2)
  Zs = sum_c exp(y[b,c]/2)            td = sum_c exp(o[b,c]/2) * (o[b,c]-y[b,c])
  m_o = max_c o[b,c]
  y[b,:] = output[b,:] / counts[t_b]        (the expert_sum_logits output)
plus the class-aggregated matrices A_k = one_hot(target)^T @ e_k in PSUM
(bf16 inputs, fp32 accumulate).  The host does the O(B)/O(C^2) assembly
(logs, means, the argmax==target mask via a gather + compare against the
device row-max, Frobenius products with the label table) in float64.
"""

import threading

import numpy as np

B = 65536
C = 100
NCORES = 8
BS = B // NCORES          # rows per core = 8192
NST = 4                   # supertiles per core
G = 16                    # 128-row blocks per supertile
ROWS_ST = 128 * G         # 2048 rows per supertile
ALPHA = 0.01
TAU = 2.0
TEMP = 2.0
EPS = 1e-9

_lock = threading.Lock()
_cache: dict = {}


def _split_waits(nc, maxw=1):
    """walrus codegen allows only a small number of sync-wait commands per
    instruction; offload extras onto preceding same-engine NoOps."""
    import concourse.mybir as mybir
    nid = 0
    for fn in nc.m.functions:
        for blk in fn.blocks:
            out = []
            changed = False
            for inst in blk.instructions:
                si = inst.sync_info
                if si is not None and si.on_wait is not None \
                        and len(si.on_wait) > maxw:
                    waits = list(si.on_wait)
                    for w in waits[maxw:]:
                        nid += 1
                        nop = mybir.InstNoOp(name=f"WSPLIT-{nid}")
                        nop.engine = inst.engine
                        nop.sync_info = mybir.SyncInfo(on_wait=[w], on_update=[])
                        out.append(nop)
                    inst.sync_info = mybir.SyncInfo(
                        on_wait=waits[:maxw],
                        on_update=list(si.on_update or []))
                    changed = True
                out.append(inst)
            if changed:
                blk.instructions = out


def _build_nc():
    import concourse.bass as bass
    import concourse.tile as tile
    import concourse.mybir as mybir
    from contextlib import ExitStack

    Op = mybir.AluOpType
    Act = mybir.ActivationFunctionType
    Ax = mybir.AxisListType
    f32 = mybir.dt.float32
    bf16 = mybir.dt.bfloat16

    nc = bass.Bass("TRN2", target_bir_lowering=False, debug=False)

    fp8 = mybir.dt.float8e4
    # packed fp8: [st, p, {e1,e2,e3,onehot}, g, c]  (feeds A_k matmuls only)
    ep = nc.dram_tensor("ep", [NST, 128, 4, G, C], fp8, kind="ExternalInput").ap()
    op_ = nc.dram_tensor("op", [NST, 128, G, C], f32, kind="ExternalInput").ap()
    u = nc.dram_tensor("u", [NST, 128, G, C], f32, kind="ExternalInput").ap()
    eT1 = nc.dram_tensor("eT1", [C, BS], bf16, kind="ExternalInput").ap()
    eT2 = nc.dram_tensor("eT2", [C, BS], bf16, kind="ExternalInput").ap()
    eT3 = nc.dram_tensor("eT3", [C, BS], bf16, kind="ExternalInput").ap()
    iv = nc.dram_tensor("iv", [NST, 128, G], f32, kind="ExternalInput").ap()
    cb = nc.dram_tensor("cb", [C, 3], f32, kind="ExternalInput").ap()

    y = nc.dram_tensor("y", [NST, 128, G, C], f32, kind="ExternalOutput").ap()
    st_out = nc.dram_tensor("stats", [128, 8, NST * G], f32, kind="ExternalOutput").ap()
    a_out = nc.dram_tensor("aout", [3, C, C], f32, kind="ExternalOutput").ap()
    lse_out = nc.dram_tensor("lse", [3, 128, NST * G], f32, kind="ExternalOutput").ap()

    with ExitStack() as ctx:
        tc = ctx.enter_context(tile.TileContext(nc))
        ins = ctx.enter_context(tc.tile_pool(name="ins", bufs=3))
        sc = ctx.enter_context(tc.tile_pool(name="sc", bufs=3))
        sm = ctx.enter_context(tc.tile_pool(name="sm", bufs=2))
        one = ctx.enter_context(tc.tile_pool(name="one", bufs=1))
        scr = ctx.enter_context(tc.tile_pool(name="scr", bufs=4))
        ps = ctx.enter_context(tc.tile_pool(name="ps", bufs=1, space="PSUM"))

        cbt = one.tile([C, 3], f32)
        nc.sync.dma_start(out=cbt[:], in_=cb)
        ones_bf = one.tile([C, 1], bf16)
        nc.vector.memset(ones_bf[:], 1.0)

        stats = one.tile([128, 8, NST * G], f32)
        a_ps = [ps.tile([C, C], f32, tag=f"a{k}", name=f"a{k}") for k in range(3)]
        lse_ps = [ps.tile([128, NST * G], f32, tag=f"l{k}", name=f"l{k}")
                  for k in range(3)]
        eT_all = (eT1, eT2, eT3)
        eT_cur = [None, None, None]

        import concourse.bass as bass_mod

        for st in range(NST):
            otl = ins.tile([128, G, C], f32, tag="otl")
            nc.sync.dma_start(out=otl[:], in_=op_[st])
            utl = ins.tile([128, G, C], f32, tag="utl")
            nc.scalar.dma_start(out=utl[:], in_=u[st])
            ot, ut = otl[:], utl[:]
            ivt = sm.tile([128, G], f32, tag="iv")
            nc.sync.dma_start(out=ivt[:], in_=iv[st])
            ept = ins.tile([128, 4, G, C], fp8, tag="ep")
            e1t, e2t, e3t, oht = (ept[:, 0], ept[:, 1], ept[:, 2], ept[:, 3])
            nc.sync.dma_start(out=ept[:], in_=ep[st])

            sl = slice(st * G, (st + 1) * G)

            tt = sc.tile([128, G, C], f32, tag="tt")
            nc.scalar.activation(tt[:], ot, Act.Exp, scale=0.5)

            # transposed experts, quarter-shard per supertile (loads
            # prefetched one supertile ahead): lse via ACT bias-exp + PE
            # per-128-row column-sum matmuls (classes on partitions)
            QB = BS // NST
            if st == 0:
                for k in range(3):
                    t0_ = ins.tile([C, QB], bf16, tag=f"eT{k}",
                                   name=f"eTt{k}_p", bufs=2)
                    nc.sync.dma_start(out=t0_[:], in_=eT_all[k][:, 0:QB])
                    eT_cur[k] = t0_
            for k in range(3):
                eTt = eT_cur[k]
                if st + 1 < NST:
                    nxt = ins.tile([C, QB], bf16, tag=f"eT{k}",
                                   name=f"eTt{k}", bufs=2)
                    nc.sync.dma_start(
                        out=nxt[:],
                        in_=eT_all[k][:, (st + 1) * QB:(st + 2) * QB])
                    eT_cur[k] = nxt
                xT = sc.tile([C, QB], bf16, tag=f"xT{k}", name=f"xT{k}",
                             bufs=2)
                if k == 0:
                    nc.scalar.activation(xT[:], eTt[:], Act.Exp)
                else:
                    nc.scalar.activation(xT[:], eTt[:], Act.Exp,
                                         bias=cbt[:, k:k + 1])
                for g2 in range(QB // 128):
                    j = st * (QB // 128) + g2
                    nc.tensor.matmul(
                        lse_ps[k][:, j:j + 1],
                        lhsT=xT[:, g2 * 128:(g2 + 1) * 128], rhs=ones_bf[:],
                        start=True, stop=True)

            # y = u * inv_num  (inv_num broadcast along the class dim) - GpSimd
            iva = ivt[:]
            iv_b = bass_mod.AP(tensor=iva.tensor, offset=iva.offset,
                               ap=[*iva.ap, [0, C]])
            yt = sc.tile([128, G, C], f32, tag="yt")
            nc.gpsimd.tensor_tensor(out=yt[:], in0=ut, in1=iv_b, op=Op.mult)
            nc.sync.dma_start(out=y[st], in_=yt[:])

            ey = sc.tile([128, G, C], f32, tag="ey")
            nc.scalar.activation(ey[:], yt[:], Act.Exp, scale=0.5)
            dtl = sc.tile([128, G, C], f32, tag="dtl", bufs=1)
            nc.vector.tensor_tensor(out=dtl[:], in0=ot, in1=yt[:],
                                    op=Op.subtract)

            # plain per-row reductions (3D AP: reduce innermost C)
            nc.vector.tensor_reduce(out=stats[:, 3, sl], in_=tt[:], axis=Ax.X, op=Op.add)
            nc.vector.tensor_reduce(out=stats[:, 4, sl], in_=ey[:], axis=Ax.X, op=Op.add)
            nc.vector.tensor_reduce(out=stats[:, 7, sl], in_=ot, axis=Ax.X, op=Op.max)

            for g in range(G):
                col = slice(st * G + g, st * G + g + 1)
                scrap = scr.tile([128, C], f32, tag="scrap")
                nc.vector.scalar_tensor_tensor(
                    out=scrap[:], in0=tt[:, g, :], scalar=1.0, in1=dtl[:, g, :],
                    op0=Op.mult, op1=Op.mult, accum_out=stats[:, 5, col])

                first = (st == 0 and g == 0)
                last = (st == NST - 1 and g == G - 1)
                nc.tensor.matmul(a_ps[0][:], lhsT=oht[:, g, :], rhs=e1t[:, g, :],
                                 start=first, stop=last)
                nc.tensor.matmul(a_ps[1][:], lhsT=oht[:, g, :], rhs=e2t[:, g, :],
                                 start=first, stop=last)
                nc.tensor.matmul(a_ps[2][:], lhsT=oht[:, g, :], rhs=e3t[:, g, :],
                                 start=first, stop=last)

        for k in range(3):
            a_sb = sm.tile([C, C], f32, tag="asb")
            nc.scalar.copy(out=a_sb[:], in_=a_ps[k][:])
            nc.sync.dma_start(out=a_out[k], in_=a_sb[:])
            l_sb = sm.tile([128, NST * G], f32, tag="lsb", name=f"lsb{k}")
            nc.scalar.copy(out=l_sb[:], in_=lse_ps[k][:])
            nc.sync.dma_start(out=lse_out[k], in_=l_sb[:])
        nc.sync.dma_start(out=st_out, in_=stats[:])

    return nc


def _get_nc():
    # hardware path: wait-split applied (walrus codegen requirement);
    # CoreSim users should call _build_nc() directly.
    with _lock:
        if "nc" not in _cache:
            nc = _build_nc()
            _split_waits(nc)
            _cache["nc"] = nc
        return _cache["nc"]


def _make_in_maps(expert1_logits, expert2_logits, expert3_logits, old_pred,
                  output, target, prior):
    import ml_dtypes
    f32 = np.float32
    bf = ml_dtypes.bfloat16
    f8 = ml_dtypes.float8_e4m3
    e1 = np.asarray(expert1_logits, dtype=f32).astype(bf)
    e2 = np.asarray(expert2_logits, dtype=f32).astype(bf)
    e3 = np.asarray(expert3_logits, dtype=f32).astype(bf)
    op_ = np.ascontiguousarray(old_pred, dtype=f32)
    u = np.ascontiguousarray(output, dtype=f32)
    tgt = np.asarray(target).astype(np.int64)
    prior = np.asarray(prior, dtype=f32)

    counts = np.bincount(tgt, minlength=C).astype(f32)
    inv_num = (1.0 / counts.astype(np.float64)).astype(f32)[tgt]  # [B]
    onehot = np.zeros((B, C), dtype=f8)
    onehot[np.arange(B), tgt] = 1

    # per-class lse offsets: c1 = 0, c2 = log(prior+EPS),
    # c3 = c2 - TAU*log(inv_prior+EPS), inv_prior = mirrored-rank prior
    p64 = prior.astype(np.float64)
    idx0 = np.argsort(p64, kind="stable")
    value = p64[idx0]
    idx1 = np.argsort(idx0, kind="stable")
    inv_prior = value[C - 1 - idx1]
    c2 = np.log(p64 + EPS)
    c3 = c2 - TAU * np.log(inv_prior + EPS)
    cb = np.stack([np.zeros(C), c2, c3], axis=1).astype(f32)  # [C, 3]

    # host side of the KL mask: o[b, target_b] (device supplies the row max)
    g_o = np.ascontiguousarray(op_[np.arange(B), tgt])

    ep_full = np.stack(
        [e1.astype(f8).reshape(NCORES, NST, 128, G, C),
         e2.astype(f8).reshape(NCORES, NST, 128, G, C),
         e3.astype(f8).reshape(NCORES, NST, 128, G, C),
         onehot.reshape(NCORES, NST, 128, G, C)],
        axis=3)  # [cores, NST, 128, 4, G, C]
    in_maps = []
    for m in range(NCORES):
        s = slice(m * BS, (m + 1) * BS)
        in_maps.append({
            "ep": ep_full[m],
            "op": op_[s].reshape(NST, 128, G, C),
            "u": u[s].reshape(NST, 128, G, C),
            "iv": inv_num[s].reshape(NST, 128, G),
            "eT1": np.ascontiguousarray(e1[s].T),
            "eT2": np.ascontiguousarray(e2[s].T),
            "eT3": np.ascontiguousarray(e3[s].T),
            "cb": cb,
        })
    aux = {"counts": counts, "target": tgt, "prior": prior,
           "inv_prior": inv_prior, "p64": p64, "g_o": g_o}
    return in_maps, aux


def _run_device(in_maps, trace=False):
    from concourse.bass_utils import run_bass_kernel_spmd
    nc = _get_nc()
    return run_bass_kernel_spmd(nc, in_maps, list(range(NCORES)), trace=trace)


def _stats_rows(stats_core):
    # stats_core: [128, 8, NST*G] -> [8, BS] with row = st*2048 + p*16 + g
    a = stats_core.reshape(128, 8, NST, G)
    return np.transpose(a, (1, 2, 0, 3)).reshape(8, BS)


def _assemble(results, aux, cos_feature):
    f64 = np.float64
    counts = aux["counts"].astype(f64)
    p64 = aux["p64"]

    # per-row stats across all cores
    allst = np.concatenate([_stats_rows(r["stats"]) for r in results], axis=1)
    _, _, _, Zt, Zs, td, _unused, m_o = [allst[i].astype(f64) for i in range(8)]
    # lse sums: [3, 128, 64], entry (m, j) is shard row j*128+m
    Sall = np.concatenate(
        [r["lse"].transpose(0, 2, 1).reshape(3, BS).astype(f64)
         for r in results], axis=1)
    S1, S2, S3 = Sall[0], Sall[1], Sall[2]

    A = np.zeros((3, C, C), dtype=f64)
    for r in results:
        A += r["aout"].astype(f64)

    # label table and per-class constants (float64)
    cosf = np.asarray(cos_feature, dtype=np.float32).astype(f64)
    sim = np.exp(cosf - cosf.max(axis=1, keepdims=True))
    sim /= sim.sum(axis=1, keepdims=True)
    L = (1.0 - ALPHA) * np.eye(C) + ALPHA * sim
    c1 = np.zeros(C)
    c2 = np.log(p64 + EPS)
    c3 = c2 - TAU * np.log(aux["inv_prior"] + EPS)

    loss = 0.0
    for k, ck in enumerate((c1, c2, c3)):
        Sk = (S1, S2, S3)[k]
        lse_mean = np.log(Sk).mean()
        dotsum = (L * A[k]).sum() + (counts * (L @ ck)).sum()
        loss += lse_mean - dotsum / B

    mask = (aux["g_o"].astype(f64) == m_o)
    n_sel = mask.sum()
    kl_per = td / (2.0 * Zt) - np.log(Zt) + np.log(Zs)
    if n_sel > 0:
        kl = (kl_per * mask).sum() / max(n_sel, 1.0) * (TEMP ** 2) * 3.0
    else:
        kl = 0.0

    y_full = np.concatenate(
        [r["y"].reshape(BS, C) for r in results], axis=0)
    return (np.float32(loss), np.float32(kl), np.ascontiguousarray(y_full))


def kernel(output_logits=None, target=None, cos_feature=None, old_pred=None,
           expert1_logits=None, expert2_logits=None, expert3_logits=None,
           output=None, prior=None, epoch=None, **_ignored):
    in_maps, aux = _make_in_maps(expert1_logits, expert2_logits,
                                 expert3_logits, old_pred, output, target,
                                 prior)
    res = _run_device(in_maps)
    return _assemble(res.results, aux, cos_feature)
